# revision 2
# baseline (speedup 1.0000x reference)
"""Correlation-volume kernel for Trainium2 (8 NeuronCores, SPMD).

Problem: inputs (B=4, N=2, C=128, H=128, W=128) fp32.
  q = floor(inputs * 1e10) / 1e10  (straight-through quantization, fp32)
  src = q[:, 0], tgt = q[:, 1]
  out[b, dy*21+dx, h, w] = mean_c src[b,c,h,w] * tgt[b,c,h+dy-10,w+dx-10]
  (zero padding outside), out shape (4, 441, 128, 128) fp32.

Strategy (v2):
  - Shard batch(4) x H-half(2) across 8 cores, data parallel, no collectives.
  - Host precomputes q, casts to bf16, pre-blocks src into 128-pixel
    stationary tiles (16 h x 8 w), zero-pads tgt; one packed input per core.
  - Device: per block, 2 bf16 matmuls (K=C=128, M=128 pixels,
    N=18 tgt rows x 28 tgt cols = 504) -> PSUM fp32; DVE/ACT copies cast to
    bf16 into a per-group staging tile (8 blocks per group).
  - Zoned dump: partition group h_l (8 partitions) only needs Gram t-rows
    h_l..h_l+20; 8 zones of 16 partitions (h_l pair 2z,2z+1) dump rows
    2z..2z+22 only (22x28 = 616 of 1008 cols): 1.40x the true output volume.
  - Input spread over sync + gpsimd + scalar DGE rings, chunk boundaries
    aligned to per-band matmul dependency ranges, ordered by first use so
    block 0's matmuls start as early as possible.
  - Zone dumps alternate sync (HWDGE) / gpsimd (SWDGE).  Tile deps are
    range-granular, so compute overlaps both the remaining input load and
    the previous groups' dumps.
  - PSUM: one 2-bank tile per block, 4-deep pool (all 8 banks), freed by a
    single strided DVE/ACT copy alternating per block; staging pool 3-deep.
  - Host extracts the valid (dy, dx) band with a single strided view
    (the per-partition "skew" is unexpressible by on-chip engines; numpy
    does it free).
"""

import sys

if "/opt/trn_rl_repo" not in sys.path:
    sys.path.insert(0, "/opt/trn_rl_repo")

import numpy as np

B, NIN, C, H, W = 4, 2, 128, 128, 128
KH = KW = 21
QS = np.float32(1e10)
HHALF = 64            # rows per core
HB, WB = 16, 8        # pixel block on stationary (M = 128)
NHB, NWB = HHALF // HB, W // WB      # 4, 16
RN2 = 18              # target rows per matmul (2 matmuls -> 36 = HB + 20)
WN = WB + 20          # 28 target cols per block
TROWS, TCOLS = HHALF + 20, W + 20    # 84, 148 padded target per core
SRC_F = HHALF * W                    # 8192
TGT_F = TROWS * TCOLS                # 12432
PACK_F = SRC_F + TGT_F
NBLK = NHB * NWB                     # 64
GRP = 8                              # blocks per staging group
NGRP = NBLK // GRP                   # 8
NZ = 8                               # partition zones per dump
ZP = 128 // NZ                       # 16 partitions per zone
ZROWS = 22                           # t-rows per zone (2 h_l + 20)
ZCOLS = ZROWS * WN                   # 616
BLKF = 2 * RN2 * WN                  # 1008 staged cols per block

_nc_cache = None


def _build_nc():
    from contextlib import ExitStack

    from concourse import bacc, mybir, tile
    from concourse._compat import with_exitstack

    nc = bacc.Bacc("TRN2")
    dt_mm = mybir.dt.bfloat16
    dt_dump = mybir.dt.bfloat16
    pack = nc.declare_dram_parameter("pack", [C, PACK_F], dt_mm, isOutput=False)
    out = nc.declare_dram_parameter(
        "out", [NGRP, NZ, ZP, GRP, ZCOLS], dt_dump, isOutput=True
    )

    # input chunks, spread over the sync + gpsimd + scalar rings, ordered
    # by first use; tgt chunk boundaries align with per-band matmul rhs
    # ranges (band hb reads tile rows 16hb..16hb+36).
    def tgt_rng(t0, t1):
        return (SRC_F + t0 * TCOLS, SRC_F + t1 * TCOLS)

    def src_rng(b0, b1):
        return (b0 * 128, b1 * 128)

    sync_chunks = [
        src_rng(0, 4), tgt_rng(18, 36), src_rng(16, 28), tgt_rng(52, 68),
        src_rng(48, 64),
    ]
    gp_chunks = [
        tgt_rng(0, 18), src_rng(8, 16), tgt_rng(36, 52), src_rng(28, 48),
    ]
    sc_chunks = [
        src_rng(4, 8), tgt_rng(68, 84),
    ]

    @with_exitstack
    def kern(ctx: ExitStack, tc: tile.TileContext):
        nc = tc.nc
        sbp = ctx.enter_context(tc.tile_pool(name="inp", bufs=1))
        psp = ctx.enter_context(tc.tile_pool(name="psp", bufs=4, space="PSUM"))
        stp = ctx.enter_context(tc.tile_pool(name="stp", bufs=3))

        pk = sbp.tile([C, PACK_F], dt_mm, tag="pk")
        for lo, hi in sync_chunks:
            nc.sync.dma_start(pk[:, lo:hi], pack[:, lo:hi])
        for lo, hi in gp_chunks:
            nc.gpsimd.dma_start(pk[:, lo:hi], pack[:, lo:hi])
        for lo, hi in sc_chunks:
            nc.scalar.dma_start(pk[:, lo:hi], pack[:, lo:hi])
        data = pk

        src2 = data[:, 0:SRC_F]
        tgt3 = data[:, SRC_F:].rearrange("c (t v) -> c t v", t=TROWS)

        for g in range(NGRP):
            sAB = stp.tile([128, GRP * BLKF], dt_dump)
            for k in range(GRP):
                blk = g * GRP + k
                hb, wb = divmod(blk, NWB)
                t0, w0 = hb * HB, wb * WB
                lhs = src2[:, blk * 128 : (blk + 1) * 128]
                # one 2-bank PSUM tile per block (504-col matmuls at 512-col
                # bank-aligned offsets); freed by a single strided copy
                ps = psp.tile([128, 1024], mybir.dt.float32)
                nc.tensor.matmul(
                    ps[:, 0:504], lhs, tgt3[:, t0 : t0 + RN2, w0 : w0 + WN],
                    start=True, stop=True,
                )
                nc.tensor.matmul(
                    ps[:, 512:1016], lhs,
                    tgt3[:, t0 + RN2 : t0 + 2 * RN2, w0 : w0 + WN],
                    start=True, stop=True,
                )
                c0 = k * BLKF
                csrc = ps[:].rearrange("p (j c) -> p j c", j=2)[:, :, 0:504]
                cdst = sAB[:, c0 : c0 + BLKF].rearrange(
                    "p (j c) -> p j c", j=2
                )
                ceng = nc.vector if blk % 2 == 0 else nc.scalar
                if ceng is nc.vector:
                    ceng.tensor_copy(cdst, csrc)
                else:
                    ceng.copy(cdst, csrc)
            for z in range(NZ):
                zsrc = (
                    sAB[z * ZP : (z + 1) * ZP, :]
                    .rearrange("p (k c) -> p k c", k=GRP)
                    [:, :, 2 * z * WN : 2 * z * WN + ZCOLS]
                )
                eng = nc.sync if z % 2 == 0 else nc.gpsimd
                eng.dma_start(out[g, z], zsrc)

    with tile.TileContext(nc) as tc:
        kern(tc)
    nc.finalize()
    return nc


def _get_nc():
    global _nc_cache
    if _nc_cache is None:
        _nc_cache = _build_nc()
    return _nc_cache


def _pack_inputs(q: np.ndarray) -> list[dict]:
    """Per-core packed input: blocked src + zero-padded tgt, bf16."""
    import ml_dtypes

    in_maps = []
    for core in range(8):
        b, half = core // 2, core % 2
        h0 = half * HHALF
        src = q[b, 0, :, h0 : h0 + HHALF, :]            # (C, 64, 128)
        srcb = (
            src.reshape(C, NHB, HB, NWB, WB)
            .transpose(0, 1, 3, 2, 4)                   # (C, hb, wb, h_l, w_l)
            .reshape(C, SRC_F)
        )
        tgt = np.zeros((C, TROWS, TCOLS), np.float32)
        lo, hi = h0 - 10, h0 + HHALF + 10
        clo, chi = max(lo, 0), min(hi, H)
        tgt[:, clo - lo : chi - lo, 10 : 10 + W] = q[b, 1, :, clo:chi, :]
        pack = np.concatenate([srcb, tgt.reshape(C, TGT_F)], axis=1)
        in_maps.append(
            {"pack": np.ascontiguousarray(pack).astype(ml_dtypes.bfloat16)}
        )
    return in_maps


def _unscramble(results: list[dict]) -> np.ndarray:
    """Extract the valid (dy, dx) band from each core's zoned Gram dump."""
    out = np.empty((B, KH * KW, H, W), np.float32)
    for core in range(8):
        b, half = core // 2, core % 2
        h0 = half * HHALF
        arr = np.asarray(results[core]["out"])
        if arr.dtype != np.float32:
            arr = arr.astype(np.float32)
        # [g, z, pp, k, col] with pp = hlz*8 + wl, col = (hlz + dy)*28 + wl + dx
        arr = np.ascontiguousarray(arr.reshape(NGRP, NZ, ZP, GRP, ZCOLS))
        s_g, s_z, s_pp, s_k, s_c = arr.strides
        gpr = NWB // GRP             # groups per hb row
        V = np.lib.stride_tricks.as_strided(
            arr,
            shape=(NHB, gpr, GRP, NZ, 2, WB, KH, KW),
            # dims: hb, wbh, wbl, z, hlz, wl, dy, dx
            strides=(
                gpr * s_g, s_g, s_k, s_z,
                8 * s_pp + WN * s_c, s_pp + s_c, WN * s_c, s_c,
            ),
        )
        # -> (dy, dx, hb, z, hlz, wbh, wbl, wl)
        oc = V.transpose(6, 7, 0, 3, 4, 1, 2, 5).reshape(KH * KW, HHALF, W)
        out[b, :, h0 : h0 + HHALF, :] = oc
    out *= np.float32(1.0 / C)
    return out


def _run(inputs: np.ndarray, trace: bool = False, trace_kwargs: dict | None = None):
    from concourse.bass_utils import run_bass_kernel_spmd

    x = np.asarray(inputs, dtype=np.float32)
    assert x.shape == (B, NIN, C, H, W), x.shape
    q = np.floor(x * QS) / QS        # fp32 ops, matches the jax reference
    in_maps = _pack_inputs(q)
    nc = _get_nc()
    res = run_bass_kernel_spmd(
        nc, in_maps, core_ids=list(range(8)), trace=trace,
        **(trace_kwargs or {}),
    )
    out = _unscramble(res.results)
    return out, res


def kernel(inputs: np.ndarray) -> np.ndarray:
    out, _ = _run(inputs, trace=False)
    return out


# revision 5
# speedup vs baseline: 1.0636x; 1.0636x over previous
"""Correlation-volume kernel for Trainium2 (8 NeuronCores, SPMD).

Problem: inputs (B=4, N=2, C=128, H=128, W=128) fp32.
  q = floor(inputs * 1e10) / 1e10  (straight-through quantization, fp32)
  src = q[:, 0], tgt = q[:, 1]
  out[b, dy*21+dx, h, w] = mean_c src[b,c,h,w] * tgt[b,c,h+dy-10,w+dx-10]
  (zero padding outside), out shape (4, 441, 128, 128) fp32.

Strategy (v2):
  - Shard batch(4) x H-half(2) across 8 cores, data parallel, no collectives.
  - Host precomputes q, casts to bf16, pre-blocks src into 128-pixel
    stationary tiles (16 h x 8 w), zero-pads tgt; one packed input per core.
  - Device: per block, 2 bf16 matmuls (K=C=128, M=128 pixels,
    N=18 tgt rows x 28 tgt cols = 504) -> PSUM fp32; DVE/ACT copies cast to
    bf16 into a per-group staging tile (8 blocks per group).
  - Zoned dump: partition group h_l (8 partitions) only needs Gram t-rows
    h_l..h_l+20; 8 zones of 16 partitions (h_l pair 2z,2z+1) dump rows
    2z..2z+22 only (22x28 = 616 of 1008 cols): 1.40x the true output volume.
  - Input spread over sync + gpsimd + scalar DGE rings, chunk boundaries
    aligned to per-band matmul dependency ranges, ordered by first use so
    block 0's matmuls start as early as possible.
  - Zone dumps alternate sync (HWDGE) / gpsimd (SWDGE).  Tile deps are
    range-granular, so compute overlaps both the remaining input load and
    the previous groups' dumps.
  - PSUM: one 2-bank tile per block, 4-deep pool (all 8 banks), freed by a
    single strided DVE/ACT copy alternating per block; staging pool 3-deep.
  - Host extracts the valid (dy, dx) band with a single strided view
    (the per-partition "skew" is unexpressible by on-chip engines; numpy
    does it free).
"""

import sys

if "/opt/trn_rl_repo" not in sys.path:
    sys.path.insert(0, "/opt/trn_rl_repo")

import numpy as np

B, NIN, C, H, W = 4, 2, 128, 128, 128
KH = KW = 21
QS = np.float32(1e10)
HHALF = 64            # rows per core
HB, WB = 16, 8        # pixel block on stationary (M = 128)
NHB, NWB = HHALF // HB, W // WB      # 4, 16
RN2 = 18              # target rows per matmul (2 matmuls -> 36 = HB + 20)
WN = WB + 20          # 28 target cols per block
TROWS, TCOLS = HHALF + 20, W + 20    # 84, 148 padded target per core
SRC_F = HHALF * W                    # 8192
TGT_F = TROWS * TCOLS                # 12432
PACK_F = SRC_F + TGT_F
NBLK = NHB * NWB                     # 64
GRP = 8                              # blocks per staging group
NGRP = NBLK // GRP                   # 8
NZ = 8                               # partition zones per dump
ZP = 128 // NZ                       # 16 partitions per zone
ZROWS = 22                           # t-rows per zone (2 h_l + 20)
ZCOLS = ZROWS * WN                   # 616
BLKF = 2 * RN2 * WN                  # 1008 staged cols per block

_nc_cache = None


def _build_nc():
    from contextlib import ExitStack

    from concourse import bacc, mybir, tile
    from concourse._compat import with_exitstack

    nc = bacc.Bacc("TRN2")
    dt_mm = mybir.dt.bfloat16
    dt_dump = mybir.dt.bfloat16
    pack = nc.declare_dram_parameter("pack", [C, PACK_F], dt_mm, isOutput=False)
    out = nc.declare_dram_parameter(
        "out", [NGRP, NZ, ZP, ZROWS, GRP, WN], dt_dump, isOutput=True
    )

    # input chunks, spread over the sync + gpsimd + scalar rings, ordered
    # by first use; tgt chunk boundaries align with per-band matmul rhs
    # ranges (band hb reads tile rows 16hb..16hb+36).
    def tgt_rng(t0, t1):
        return (SRC_F + t0 * TCOLS, SRC_F + t1 * TCOLS)

    def src_rng(b0, b1):
        return (b0 * 128, b1 * 128)

    sync_chunks = [
        src_rng(0, 4), tgt_rng(18, 36), src_rng(16, 28), tgt_rng(52, 68),
        src_rng(48, 64),
    ]
    gp_chunks = [
        tgt_rng(0, 18), src_rng(8, 16), tgt_rng(36, 52), src_rng(28, 48),
    ]
    sc_chunks = [
        src_rng(4, 8), tgt_rng(68, 84),
    ]

    @with_exitstack
    def kern(ctx: ExitStack, tc: tile.TileContext):
        nc = tc.nc
        sbp = ctx.enter_context(tc.tile_pool(name="inp", bufs=1))
        psp = ctx.enter_context(tc.tile_pool(name="psp", bufs=4, space="PSUM"))
        stp = ctx.enter_context(tc.tile_pool(name="stp", bufs=3))

        pk = sbp.tile([C, PACK_F], dt_mm, tag="pk")
        for lo, hi in sync_chunks:
            nc.sync.dma_start(pk[:, lo:hi], pack[:, lo:hi])
        for lo, hi in gp_chunks:
            nc.gpsimd.dma_start(pk[:, lo:hi], pack[:, lo:hi])
        for lo, hi in sc_chunks:
            nc.scalar.dma_start(pk[:, lo:hi], pack[:, lo:hi])
        data = pk

        src2 = data[:, 0:SRC_F]
        tgt3 = data[:, SRC_F:].rearrange("c (t v) -> c t v", t=TROWS)

        for g in range(NGRP):
            sAB = stp.tile([128, GRP * BLKF], dt_dump)
            for k in range(GRP):
                blk = g * GRP + k
                hb, wb = divmod(blk, NWB)
                t0, w0 = hb * HB, wb * WB
                lhs = src2[:, blk * 128 : (blk + 1) * 128]
                # one 2-bank PSUM tile per block (504-col matmuls at 512-col
                # bank-aligned offsets); freed by a single strided copy
                ps = psp.tile([128, 1024], mybir.dt.float32)
                nc.tensor.matmul(
                    ps[:, 0:504], lhs, tgt3[:, t0 : t0 + RN2, w0 : w0 + WN],
                    start=True, stop=True,
                )
                nc.tensor.matmul(
                    ps[:, 512:1016], lhs,
                    tgt3[:, t0 + RN2 : t0 + 2 * RN2, w0 : w0 + WN],
                    start=True, stop=True,
                )
                # staging layout per partition is [t_r(36), k(GRP), c(28)]
                # so each zone dump below is ONE contiguous run per
                # partition (HWDGE issue cost scales with runs/partition).
                csrc = (
                    ps[:].rearrange("p (j q) -> p j q", j=2)[:, :, 0:504]
                    .rearrange("p j (t c) -> p j t c", t=RN2)
                )
                cdst = sAB[:].rearrange(
                    "p (j t k c) -> p j t k c", j=2, t=RN2, k=GRP
                )[:, :, :, k, :]
                ceng = nc.vector if blk % 2 == 0 else nc.scalar
                if ceng is nc.vector:
                    ceng.tensor_copy(cdst, csrc)
                else:
                    ceng.copy(cdst, csrc)
            for z in range(NZ):
                zsrc = sAB[z * ZP : (z + 1) * ZP, :].rearrange(
                    "p (t k c) -> p t k c", t=2 * RN2, k=GRP
                )[:, 2 * z : 2 * z + ZROWS, :, :]
                eng = nc.sync if z % 2 == 0 else nc.gpsimd
                eng.dma_start(out[g, z], zsrc)

    with tile.TileContext(nc) as tc:
        kern(tc)
    nc.finalize()
    return nc


def _get_nc():
    global _nc_cache
    if _nc_cache is None:
        _nc_cache = _build_nc()
    return _nc_cache


def _pack_inputs(q: np.ndarray) -> list[dict]:
    """Per-core packed input: blocked src + zero-padded tgt, bf16."""
    import ml_dtypes

    in_maps = []
    for core in range(8):
        b, half = core // 2, core % 2
        h0 = half * HHALF
        src = q[b, 0, :, h0 : h0 + HHALF, :]            # (C, 64, 128)
        srcb = (
            src.reshape(C, NHB, HB, NWB, WB)
            .transpose(0, 1, 3, 2, 4)                   # (C, hb, wb, h_l, w_l)
            .reshape(C, SRC_F)
        )
        tgt = np.zeros((C, TROWS, TCOLS), np.float32)
        lo, hi = h0 - 10, h0 + HHALF + 10
        clo, chi = max(lo, 0), min(hi, H)
        tgt[:, clo - lo : chi - lo, 10 : 10 + W] = q[b, 1, :, clo:chi, :]
        pack = np.concatenate([srcb, tgt.reshape(C, TGT_F)], axis=1)
        in_maps.append(
            {"pack": np.ascontiguousarray(pack).astype(ml_dtypes.bfloat16)}
        )
    return in_maps


def _unscramble(results: list[dict]) -> np.ndarray:
    """Extract the valid (dy, dx) band from each core's zoned Gram dump."""
    out = np.empty((B, KH * KW, H, W), np.float32)
    for core in range(8):
        b, half = core // 2, core % 2
        h0 = half * HHALF
        arr = np.asarray(results[core]["out"])
        if arr.dtype != np.float32:
            arr = arr.astype(np.float32)
        # [g, z, pp, col] with pp = hlz*8 + wl,
        # col = (hlz + dy)*GRP*WN + k*WN + wl + dx  ([t_r, k, c] layout)
        arr = np.ascontiguousarray(arr.reshape(NGRP, NZ, ZP, ZROWS * GRP * WN))
        s_g, s_z, s_pp, s_c = arr.strides
        gpr = NWB // GRP             # groups per hb row
        V = np.lib.stride_tricks.as_strided(
            arr,
            shape=(NHB, gpr, GRP, NZ, 2, WB, KH, KW),
            # dims: hb, wbh, wbl(k), z, hlz, wl, dy, dx
            strides=(
                gpr * s_g, s_g, WN * s_c, s_z,
                8 * s_pp + GRP * WN * s_c, s_pp + s_c, GRP * WN * s_c, s_c,
            ),
        )
        # -> (dy, dx, hb, z, hlz, wbh, wbl, wl)
        oc = V.transpose(6, 7, 0, 3, 4, 1, 2, 5).reshape(KH * KW, HHALF, W)
        out[b, :, h0 : h0 + HHALF, :] = oc
    out *= np.float32(1.0 / C)
    return out


def _run(inputs: np.ndarray, trace: bool = False, trace_kwargs: dict | None = None):
    from concourse.bass_utils import run_bass_kernel_spmd

    x = np.asarray(inputs, dtype=np.float32)
    assert x.shape == (B, NIN, C, H, W), x.shape
    q = np.floor(x * QS) / QS        # fp32 ops, matches the jax reference
    in_maps = _pack_inputs(q)
    nc = _get_nc()
    res = run_bass_kernel_spmd(
        nc, in_maps, core_ids=list(range(8)), trace=trace,
        **(trace_kwargs or {}),
    )
    out = _unscramble(res.results)
    return out, res


def kernel(inputs: np.ndarray) -> np.ndarray:
    out, _ = _run(inputs, trace=False)
    return out
